# revision 46
# baseline (speedup 1.0000x reference)
"""Trainium2 Bass kernel for nn_NodeClassifier (2-layer hetero-RGCN, mean aggregation).

Strategy (8 NeuronCores, dst-node sharding):
  - Mean-aggregation commutes with the per-etype linear: segmean(h @ W) = segmean(h) @ W,
    so we gather RAW bf16 node rows per edge (dma_gather), segment-sum them via TensorE
    matmuls against an on-device-built 0/1 selection matrix (dst_rel == iota), apply the
    256x256 weights after aggregation, and scale rows by 1/deg at the combine stage.
  - Edges are sharded by destination-node ownership; nodes are dealt to
    (core, window-slot) bins in descending-degree stripes so per-window edge counts
    stay balanced across cores (smaller max-over-core chunk quotas).
  - Chunks are packed at GROUP-of-windows granularity and may span window
    boundaries; a chunk feeds each overlapped window through its own masked
    S column (rel values outside the window never match iota).
  - SWDGE gathers run on 4 queues; the completion-semaphore -> queue map is
    rewritten post-scheduling so every sem is driven by exactly one queue
    (ucode requirement) while adjacent calls alternate queues (desc-gen of
    the 4 Q7 queue contexts runs concurrently: ~2.7x pool throughput).
  - Per-(core, etype) packed layer-1 tables (each core's distinct src rows,
    re-indexed < 32768) kill the int16 lo/hi split for layer 1.  Layer 2
    gathers from the AllGathered h2 tables and keeps the lo/hi split.
  - Vector work is batched (one is_equal S-build per gather call); PSUM
    evacuation, 1/deg scaling (per-partition scale) and leaky-relu run on the
    otherwise-idle Activation engine.
"""
import os
import sys

for _p in ("/opt/trn_rl_repo", "/root/.axon_site/_ro/trn_rl_repo"):
    if os.path.isdir(_p) and _p not in sys.path:
        sys.path.append(_p)

import numpy as np
import ml_dtypes

import concourse.bass as bass
import concourse.bacc as bacc
import concourse.mybir as mybir
import concourse.tile as tile
from concourse.bass_utils import run_bass_kernel_spmd

BF16 = mybir.dt.bfloat16
F32 = mybir.dt.float32
I16 = mybir.dt.int16

ETYPES = [("chemical", "ch2ge", "gene"),
          ("gene", "ge2ch", "chemical"),
          ("chemical", "ch2ch", "chemical"),
          ("gene", "ge2ge", "gene")]
D = 256
NCORES = 8
LO_LIM = 32768
GROUP = int(os.environ.get("KERNEL_GROUP", "3"))   # windows per dma_gather call
NQ = int(os.environ.get("KERNEL_NQ", "4"))   # swdge queues (validated post-compile; >1 falls back unless provably race-free)


def _bf(x):
    return np.ascontiguousarray(np.asarray(x, np.float32)).astype(ml_dtypes.bfloat16)


def _wrap_idx(idx):
    """int16 idx array (len % 128 == 0) -> [128, n/16] wrapped + replicated layout."""
    n = len(idx)
    w = np.zeros((16, n // 16), np.int16)
    ar = np.arange(n)
    w[ar % 16, ar // 16] = idx
    return np.tile(w, (8, 1))


class LayerPrep:
    """Host-side per-layer gather planning with group-granular chunking.

    Edges of each (etype, half) stream are packed per GROUP of windows into
    128-row chunks that may span window boundaries; a chunk contributes to a
    window w through an S column that masks rows of other windows (rel = -1
    never matches iota).  The (chunk, window) incidence is the union over
    cores so the shared SPMD program is valid everywhere.  Produces, per
    core, the gather index / dst_rel / rdeg tensors plus the compile-time
    quota structure (max over cores)."""

    def __init__(self, n_nodes, etlist, srcs, dsts, src_row_of, n_src_rows,
                 dst_map=None):
        self.etlist = etlist
        self.slice_n = {nt: n // NCORES for nt, n in n_nodes.items()}
        self.wpc = {nt: (self.slice_n[nt] + 127) // 128 for nt in n_nodes}
        self.rows_pad = {nt: self.wpc[nt] * 128 for nt in n_nodes}
        self.gquota = {}      # (et, half) -> list[ngrp] of chunk counts
        self.gcols = {}       # (et, half) -> list[ngrp] of S-column counts
        self.win_chunks = {}  # (et, half, grp) -> {w: [(chunk_local, col_local)]}
        self.n_src_rows = n_src_rows
        self.dst_map = dst_map or {}

        percore = [dict() for _ in range(NCORES)]
        for st, et, dt in etlist:
            s, d = srcs[et], dsts[et]
            sn, wpc = self.slice_n[dt], self.wpc[dt]
            ngrp = -(-wpc // GROUP)
            if dt in self.dst_map:
                pc, pl = self.dst_map[dt]
                core_of, loc = pc[d], pl[d]
            else:
                core_of = d // sn
                loc = d - core_of * sn
            win, rel = loc // 128, loc % 128
            rows_spec = src_row_of[et]
            per_core_rows = isinstance(rows_spec, (list, tuple))
            if per_core_rows:
                half = np.zeros(len(s), np.int8)   # packed rows all < LO_LIM
                s_rows = None
            else:
                rows = rows_spec[s]
                half = (rows >= LO_LIM).astype(np.int8)
            deg = np.bincount(d, minlength=n_nodes[dt]).astype(np.float32)
            rdeg_full = 1.0 / np.maximum(deg, 1.0)

            counts = np.zeros((NCORES, wpc, 2), np.int64)
            np.add.at(counts, (core_of, win, half), 1)

            key = core_of.astype(np.int64) * (wpc * 2) + win * 2 + half
            order = np.argsort(key, kind="stable")
            s_src, s_rel = s[order], rel[order]
            if not per_core_rows:
                s_rows = rows[order]
            s_key = key[order]

            for h in (0, 1):
                if per_core_rows and h == 1:
                    self.gquota[(et, 1)] = [0] * ngrp
                    self.gcols[(et, 1)] = [0] * ngrp
                    for g in range(ngrp):
                        self.win_chunks[(et, 1, g)] = {}
                    continue
                gq, gc = [], []
                for g in range(ngrp):
                    ws = range(g * GROUP, min((g + 1) * GROUP, wpc))
                    cg = counts[:, list(ws), h].sum(axis=1)       # per core
                    nch = max(1, -(-int(cg.max()) // 128))
                    gq.append(nch)
                    # union (chunk, window) incidence over cores
                    pairs = set()
                    for c in range(NCORES):
                        off = 0
                        for w in ws:
                            cnt = int(counts[c, w, h])
                            a, b = off, off + cnt
                            j0 = min(a // 128, nch - 1)
                            j1 = min(max(b - 1, a) // 128, nch - 1)
                            for j in range(j0, j1 + 1):
                                pairs.add((j, w))
                            off = b
                    plist = sorted(pairs, key=lambda p: (p[1], p[0]))
                    gc.append(len(plist))
                    self.win_chunks[(et, h, g)] = wc = {}
                    for ci, (j, w) in enumerate(plist):
                        wc.setdefault(w, []).append((j, ci))
                self.gquota[(et, h)] = gq
                self.gcols[(et, h)] = gc

            for c in range(NCORES):
                for h in (0, 1):
                    gq = self.gquota[(et, h)]
                    nch_tot = sum(gq)
                    idx_arr = np.zeros(nch_tot * 128, np.int16)
                    ncol_tot = sum(self.gcols[(et, h)])
                    rel_mat = np.full((ncol_tot, 128), -1.0, np.float32)
                    goff = col0 = 0
                    for g in range(-(-wpc // GROUP)):
                        ws = range(g * GROUP, min((g + 1) * GROUP, wpc))
                        off = 0
                        spans = {}
                        for w in ws:
                            kk = c * (wpc * 2) + w * 2 + h
                            a = np.searchsorted(s_key, kk)
                            b = np.searchsorted(s_key, kk, side="right")
                            cnt = b - a
                            if per_core_rows:
                                vals = rows_spec[c][s_src[a:b]]
                            else:
                                vals = s_rows[a:b] - LO_LIM * h
                            idx_arr[goff + off:goff + off + cnt] = \
                                vals.astype(np.int16)
                            spans[w] = (off, off + cnt, a)
                            off += cnt
                        # fill this core's rel columns for the group's pairs
                        wc = self.win_chunks[(et, h, g)]
                        for w, jlist in wc.items():
                            o0, o1, a = spans[w]
                            for (j, ci) in jlist:
                                lo = max(o0, j * 128)
                                hi = min(o1, (j + 1) * 128)
                                if hi > lo:
                                    rel_mat[col0 + ci, lo - j * 128:hi - j * 128] = \
                                        s_rel[a + (lo - o0):a + (hi - o0)]
                        goff += gq[g] * 128
                        col0 += self.gcols[(et, h)][g]
                    percore[c][(et, h, "idx")] = idx_arr
                    percore[c][(et, h, "rel")] = rel_mat
                pad = np.ones(self.rows_pad[dt], np.float32)
                if dt in self.dst_map:
                    pc, pl = self.dst_map[dt]
                    mine = np.where(pc == c)[0]
                    pad[pl[mine]] = rdeg_full[mine]
                else:
                    pad[:sn] = rdeg_full[c * sn:c * sn + sn]
                percore[c][(et, "rdeg")] = pad.reshape(wpc, 128).T.copy()

        # stream offsets (chunks / S cols / rdeg) in the concatenated tensors
        self.chunk_off, self.col_off, self.rdeg_off = {}, {}, {}
        ch_cur = co_cur = rd_cur = 0
        for st, et, dt in etlist:
            for h in (0, 1):
                self.chunk_off[(et, h)] = ch_cur
                ch_cur += sum(self.gquota[(et, h)])
                self.col_off[(et, h)] = co_cur
                co_cur += sum(self.gcols[(et, h)])
            self.rdeg_off[et] = rd_cur
            rd_cur += self.wpc[dt]
        self.tot_chunks, self.tot_cols, self.tot_rdeg = ch_cur, co_cur, rd_cur

        self.tensors = []
        for c in range(NCORES):
            idx_cols, rdegs = [], []
            rel_mat = np.full((128, self.tot_cols), -1.0, np.float32)
            for st, et, dt in etlist:
                for h in (0, 1):
                    idx_cols.append(_wrap_idx(percore[c][(et, h, "idx")]))
                    rel = percore[c][(et, h, "rel")]
                    co = self.col_off[(et, h)]
                    rel_mat[:, co:co + rel.shape[0]] = rel.T
                rdegs.append(percore[c][(et, "rdeg")])
            self.tensors.append(dict(
                idx=np.concatenate(idx_cols, axis=1),
                rel=rel_mat.astype(ml_dtypes.bfloat16),
                rdeg=np.ascontiguousarray(np.concatenate(rdegs, axis=1)),
            ))


def _np_reference(inputs, n_nodes):
    """Pure-numpy fp32 fallback (used only when biases are nonzero)."""
    def layer(h, Wk, bk):
        agg = {nt: np.zeros((n, D), np.float32) for nt, n in n_nodes.items()}
        for st, et, dt in ETYPES:
            Wh = h[st] @ inputs[f"{Wk}_{et}"] + inputs[f"{bk}_{et}"]
            msg = Wh[inputs[f"src_{et}"]]
            ssum = np.zeros((n_nodes[dt], D), np.float32)
            np.add.at(ssum, inputs[f"dst_{et}"], msg)
            cnt = np.bincount(inputs[f"dst_{et}"], minlength=n_nodes[dt]).astype(np.float32)[:, None]
            agg[dt] += ssum / np.maximum(cnt, 1.0)
        return agg
    h = {"chemical": np.asarray(inputs["chemical_embed"], np.float32),
         "gene": np.asarray(inputs["gene_embed"], np.float32)}
    h = layer(h, "W1", "b1")
    h = {k: np.where(v > 0, v, np.float32(0.01) * v) for k, v in h.items()}
    return layer(h, "W2", "b2")["chemical"]


def _builder(nq, inputs, n_nodes, L1, L2, n_packed):
    l1_ets = ETYPES
    l2_ets = [e for e in ETYPES if e[2] == 'chemical']
    nc = bacc.Bacc("TRN2", target_bir_lowering=False, debug=False,
                   num_devices=NCORES, num_swdge_queues=nq)
    # per-(core, etype) packed source tables for layer 1 (content differs per
    # core via in_maps; all packed rows < LO_LIM so L1 has no hi stream)
    tabs = {et: nc.dram_tensor(f"ptab_{et}", [n_packed[et], D], BF16,
                               kind="ExternalInput")
            for _, et, _ in l1_ets}
    w_in = {(1, et): nc.dram_tensor(f"w1_{et}", [D, D], BF16, kind="ExternalInput")
            for _, et, _ in l1_ets}
    w_in.update({(2, et): nc.dram_tensor(f"w2_{et}", [D, D], BF16, kind="ExternalInput")
                 for _, et, _ in l2_ets})
    dram_in = {}
    for li, LP in ((1, L1), (2, L2)):
        t0 = LP.tensors[0]
        dram_in[(li, "idx")] = nc.dram_tensor(f"idx{li}", list(t0["idx"].shape), I16,
                                              kind="ExternalInput")
        dram_in[(li, "rel")] = nc.dram_tensor(f"rel{li}", list(t0["rel"].shape), BF16,
                                              kind="ExternalInput")
        dram_in[(li, "rdeg")] = nc.dram_tensor(f"rdeg{li}", list(t0["rdeg"].shape), F32,
                                               kind="ExternalInput")
    iota_t = nc.dram_tensor("iota", [128, 128], F32, kind="ExternalInput")
    out_t = nc.dram_tensor("out", [L1.rows_pad["chemical"], D], F32, kind="ExternalOutput")

    h2_slice = {nt: nc.dram_tensor(f"h2s_{nt}", [L1.rows_pad[nt], D], BF16)
                for nt in n_nodes}
    h2_full = {nt: nc.dram_tensor(f"h2f_{nt}", [L1.rows_pad[nt] * NCORES, D], BF16,
                                  addr_space="Shared")
               for nt in n_nodes}

    import contextlib
    with tile.TileContext(nc) as tc, contextlib.ExitStack() as ctx:
        const = ctx.enter_context(tc.tile_pool(name="const", bufs=1))
        iota_f = const.tile([128, 128], F32, tag="iotaf")
        iota_sb = const.tile([128, 1, 128], BF16, tag="iotab")
        nc.sync.dma_start(iota_f[:], iota_t[:])
        nc.vector.tensor_copy(iota_sb[:, 0, :], iota_f[:])
        w_sb = {}
        for key, t in w_in.items():
            # scalar-engine DMA: keeps the weight loads off the Sync FIFO so
            # the first gather's idx load isn't queued behind them
            w = const.tile([128, 2 * D], BF16, tag=f"w_{key[0]}_{key[1]}")
            nc.scalar.dma_start(w[:, 0:D], t[0:128, :])
            nc.scalar.dma_start(w[:, D:2 * D], t[128:256, :])
            w_sb[key] = w
        rel_sb, rdeg_sb = {}, {}
        for li, LP in ((1, L1), (2, L2)):
            r = const.tile(list(LP.tensors[0]["rel"].shape), BF16, tag=f"rel{li}")
            nc.sync.dma_start(r[:], dram_in[(li, "rel")][:])
            rel_sb[li] = r
            g = const.tile(list(LP.tensors[0]["rdeg"].shape), F32, tag=f"rdeg{li}")
            nc.sync.dma_start(g[:], dram_in[(li, "rdeg")][:])
            rdeg_sb[li] = g

        def do_layer(li, LP, gtab, sink, ntype_done=None, prepass_et=None):
            etlist = LP.etlist
            with contextlib.ExitStack() as lctx:
                gp, ip, sp = {}, {}, {}

                def open_stream_pools(stack, ets):
                    for st, et, dt in ets:
                        for h in (0, 1):
                            if sum(LP.gquota[(et, h)]) == 0:
                                continue
                            nb = 3 if (et == prepass_et and h == 0) else \
                                int(os.environ.get("KERNEL_L2BUFS", "2")) \
                                if (li == 2 and h == 0) else 2
                            gp[(et, h)] = stack.enter_context(
                                tc.tile_pool(name=f"g{li}{et}{h}", bufs=nb))
                            ip[(et, h)] = stack.enter_context(
                                tc.tile_pool(name=f"i{li}{et}{h}", bufs=2))
                            sp[(et, h)] = stack.enter_context(
                                tc.tile_pool(name=f"s{li}{et}{h}", bufs=2))
                mt_psum = lctx.enter_context(
                    tc.tile_pool(name=f"mtp{li}", bufs=2, space="PSUM"))
                agg_psum = lctx.enter_context(
                    tc.tile_pool(name=f"agp{li}", bufs=2, space="PSUM"))
                mt_pool = lctx.enter_context(tc.tile_pool(name=f"mt{li}", bufs=3))
                cb_pool = lctx.enter_context(tc.tile_pool(name=f"cb{li}", bufs=4))

                call_tiles = {}

                def compute_mt(et, grp, w, pool):
                    """Segment-sum the window's chunks into PSUM, copy to SBUF bf16."""
                    chunks = []
                    for h in (0, 1):
                        for (j, cl) in LP.win_chunks[(et, h, grp)].get(w, []):
                            chunks.append((h, j, cl))
                    mt0 = mt_psum.tile([128, 128], F32, tag="mt0")
                    mt1 = mt_psum.tile([128, 128], F32, tag="mt1")
                    mts = [mt0, mt1]
                    nlast = len(chunks) - 1
                    for ci, (h, ch_l, cl) in enumerate(chunks):
                        gt, stt, _w0 = call_tiles[(et, h, grp)]
                        for fh in (0, 1):
                            nc.tensor.matmul(
                                mts[fh][:],
                                lhsT=gt[:, ch_l, fh * 128:(fh + 1) * 128],
                                rhs=stt[:, cl, :],
                                start=(ci == 0), stop=(ci == nlast))
                    mt_sb = pool.tile([128, 2 * 128], BF16, tag="mtsb")
                    nc.scalar.activation(mt_sb[:, 0:128], mts[0][:],
                                         mybir.ActivationFunctionType.Copy)
                    nc.scalar.activation(mt_sb[:, 128:256], mts[1][:],
                                         mybir.ActivationFunctionType.Copy)
                    return mt_sb

                def wapply(et, mt_sb):
                    ag = agg_psum.tile([128, D], F32, tag="agg")
                    for fh in (0, 1):
                        nc.tensor.matmul(
                            ag[:],
                            lhsT=mt_sb[:, fh * 128:(fh + 1) * 128],
                            rhs=w_sb[(li, et)][:, fh * D:(fh + 1) * D],
                            start=(fh == 0), stop=(fh == 1))
                    return ag

                def issue_call(et, h, dt, grp, qnum):
                    gq = LP.gquota[(et, h)]
                    w0 = grp * GROUP
                    nch = gq[grp]
                    if nch == 0:
                        return
                    chunk_base = LP.chunk_off[(et, h)] + sum(gq[:grp])
                    icol0 = chunk_base * 8          # 128 idx per chunk / 16
                    it = ip[(et, h)].tile([128, nch * 8], I16, tag=f"it{et}{h}")
                    nc.sync.dma_start(it[:], dram_in[(li, "idx")][:, icol0:icol0 + nch * 8])
                    gt = gp[(et, h)].tile([128, nch, D], BF16, tag=f"gt{et}{h}")
                    tab, nrows = gtab[et]
                    base = LO_LIM * h
                    if base >= nrows:
                        base = 0    # half has no real rows; pads (idx 0) only
                    view = tab[base:min(base + LO_LIM, nrows), :]
                    nc.gpsimd.dma_gather(
                        out_ap=gt[:], in_ap=view, idxs_ap=it[:],
                        num_idxs=nch * 128, num_idxs_reg=nch * 128,
                        elem_size=D, single_packet=False, queue_num=qnum)
                    # one batched S-build for the whole call: [128, ncols, 128]
                    ncols = LP.gcols[(et, h)][grp]
                    scol0 = LP.col_off[(et, h)] + sum(LP.gcols[(et, h)][:grp])
                    stt = sp[(et, h)].tile([128, ncols, 128], BF16, tag=f"st{et}{h}")
                    nc.vector.tensor_tensor(
                        out=stt[:],
                        in0=rel_sb[li][:, scol0:scol0 + ncols]
                            .to_broadcast([128, ncols, 128]),
                        in1=iota_sb[:].to_broadcast([128, ncols, 128]),
                        op=mybir.AluOpType.is_equal)
                    call_tiles[(et, h, grp)] = (gt, stt, w0)

                ntypes = sorted({dt for _, _, dt in etlist}, reverse=True)  # gene first: its AllGather overlaps chemical-dst compute
                for nt in ntypes:
                    my_ets = [e for e in etlist if e[2] == nt]
                    with contextlib.ExitStack() as pctx:
                        open_stream_pools(pctx, my_ets)
                        wpc = LP.wpc[nt]
                        ngrp = -(-wpc // GROUP)
                        pre_mts = {}
                        if prepass_et and any(et == prepass_et
                                              for _, et, _ in my_ets):
                            # Pass A: gather + segment-sum for prepass_et only.
                            # Its source table is ready before the other
                            # etypes' (it only depends on the earlier
                            # AllGather), so this work fills the wait for the
                            # later AllGather.
                            pmt_pool = pctx.enter_context(
                                tc.tile_pool(name=f"pm{li}", bufs=wpc))
                            for grp in range(ngrp):
                                for h in (0, 1):
                                    issue_call(prepass_et, h, nt, grp, 0)
                                for w in range(grp * GROUP,
                                               min((grp + 1) * GROUP, wpc)):
                                    pre_mts[w] = compute_mt(prepass_et, grp, w,
                                                            pmt_pool)
                            my_ets = [e for e in my_ets
                                      if e[1] != prepass_et]
                        for grp in range(ngrp):
                            for st_, et, _ in my_ets:
                                for h in (0, 1):
                                    issue_call(et, h, nt, grp, 0)
                            for w in range(grp * GROUP,
                                           min((grp + 1) * GROUP, wpc)):
                                aggs = []
                                if w in pre_mts:
                                    aggs.append((prepass_et,
                                                 wapply(prepass_et,
                                                        pre_mts[w])))
                                for st_, et, _ in my_ets:
                                    mt_sb = compute_mt(et, grp, w, mt_pool)
                                    aggs.append((et, wapply(et, mt_sb)))
                                sink(nt, w, aggs, cb_pool, li, LP)
                    if ntype_done:
                        ntype_done(nt)

        def combine(nt, w, aggs, cb_pool, li, LP):
            acc = None
            for et, ag in aggs:
                col = LP.rdeg_off[et] + w
                t = cb_pool.tile([128, D], F32, tag="cbt")
                nc.scalar.activation(t[:], ag[:],
                                     mybir.ActivationFunctionType.Copy,
                                     scale=rdeg_sb[li][:, col:col + 1])
                if acc is None:
                    acc = t
                else:
                    t2 = cb_pool.tile([128, D], F32, tag="cbt")
                    nc.vector.tensor_tensor(out=t2[:], in0=acc[:], in1=t[:],
                                            op=mybir.AluOpType.add)
                    acc = t2
            if li == 1:
                h2w = cb_pool.tile([128, D], BF16, tag="h2w")
                nc.scalar.activation(h2w[:], acc[:],
                                     mybir.ActivationFunctionType.Lrelu,
                                     alpha=0.01)
                nc.sync.dma_start(h2_slice[nt][w * 128:(w + 1) * 128, :], h2w[:])
            else:
                nc.sync.dma_start(out_t[w * 128:(w + 1) * 128, :], acc[:])

        def ag_ntype(nt):
            nc.gpsimd.collective_compute(
                "AllGather", mybir.AluOpType.bypass,
                replica_groups=[list(range(NCORES))],
                ins=[h2_slice[nt].ap().opt()],
                outs=[h2_full[nt].ap().opt()])

        gtab1 = {et: (tabs[et], n_packed[et]) for st, et, _ in l1_ets}
        gtab2 = {et: (h2_full[st], L1.rows_pad[st] * NCORES) for st, et, _ in l2_ets}
        do_layer(1, L1, gtab1, combine, ntype_done=ag_ntype)
        do_layer(2, L2, gtab2, combine)

    if nq > 1:
        # Re-derive each gather's queue from its scheduler-assigned completion
        # semaphore: sem -> queue is then 1:1 (the ucode constraint) and
        # per-sem increments stay in-order (same queue => FIFO).  The map
        # keeps consecutive sem lanes on different queues (adjacent calls
        # pipeline across queues) while pairing each heavy lo-half lane with
        # a light hi-half lane per queue for byte balance.
        lane_q = {0: 0, 1: 1, 2: 2, 3: 3, 4: 1, 5: 0, 6: 3, 7: 2}
        gathers = []
        for bb in nc.m.functions[0].blocks:
            for ins in bb.instructions:
                if isinstance(ins, mybir.InstDMAGatherAnt) and ins.sync_info:
                    sems = [u.id for u in ins.sync_info.on_update
                            if u.sync_type == "semaphore"]
                    if sems:
                        gathers.append((ins, sems[0]))
        order = {s: i for i, s in enumerate(dict.fromkeys(s for _, s in gathers))}
        for ins, s in gathers:
            ins.queue_num = lane_q[order[s] % 8] % nq

    nc.compile()

    return nc


def run(inputs, n_nodes):
    srcs = {et: np.asarray(inputs[f"src_{et}"]) for _, et, _ in ETYPES}
    dsts = {et: np.asarray(inputs[f"dst_{et}"]) for _, et, _ in ETYPES}
    l1_ets = ETYPES
    l2_ets = [e for e in ETYPES if e[2] == "chemical"]

    # Striped dst-sharding: deal degree-sorted nodes round-robin across
    # (core, window-slot) so per-(etype, window) edge counts flatten across
    # cores and windows -> smaller max-over-cores chunk quotas.
    dst_map = {}
    for nt, n in n_nodes.items():
        deg = np.zeros(n, np.int64)
        for _, et, dt in ETYPES:
            if dt == nt:
                deg += np.bincount(dsts[et], minlength=n)
        order = np.argsort(-deg, kind="stable")
        pc = np.empty(n, np.int64)
        pl = np.empty(n, np.int64)
        r = np.arange(n)
        pc[order] = r % NCORES
        pl[order] = r // NCORES
        dst_map[nt] = (pc, pl)

    # layer-1 gathers read the raw embed tables at node index (lo/hi halves;
    # per-core packed tables measured slower: coarser calls pipeline worse)
    tab_bf = {"chemical": _bf(inputs["chemical_embed"]), "gene": _bf(inputs["gene_embed"])}
    ident = {nt: np.arange(n, dtype=np.int64) for nt, n in n_nodes.items()}
    row_of1 = {et: ident[st] for st, et, _ in l1_ets}
    n_packed = {et: n_nodes[st] for st, et, _ in l1_ets}
    L1 = LayerPrep(n_nodes, l1_ets, srcs, dsts, row_of1, dict(n_nodes),
                   dst_map=dst_map)

    # layer-2 rows live in the AllGathered (per-core padded) tables
    row_of2, n_rows2 = {}, {}
    for nt in n_nodes:
        pc, pl = dst_map[nt]
        row_of2[nt] = pc * L1.rows_pad[nt] + pl
        n_rows2[nt] = L1.rows_pad[nt] * NCORES
    L2 = LayerPrep(n_nodes, l2_ets, srcs, dsts,
                   {et: row_of2[st] for st, et, _ in l2_ets}, n_rows2,
                   dst_map=dst_map)
    iota = np.tile(np.arange(128, dtype=np.float32)[None, :], (128, 1))

    def _swdge_queues_ok(nc_):
        """Each SWDGE completion semaphore must be driven by exactly one queue
        (ucode locks a sem to the first queue that uses it)."""
        qmap = {}
        for bb in nc_.m.functions[0].blocks:
            for ins in bb.instructions:
                if isinstance(ins, mybir.InstDMAGatherAnt) and ins.sync_info:
                    for u in ins.sync_info.on_update:
                        if u.sync_type == "semaphore":
                            qmap.setdefault(u.id, set()).add(ins.queue_num)
        return all(len(v) == 1 for v in qmap.values())

    nc = None
    for nq_try in [NQ] + [q for q in (2, 1) if q < NQ]:
        nc = _builder(nq_try, inputs, n_nodes, L1, L2, n_packed)
        if _swdge_queues_ok(nc):
            print(f"[kernel] using num_swdge_queues={nq_try}")
            break
        print(f"[kernel] queue collision at nq={nq_try}, falling back")
    assert nc is not None

    in_maps = []
    for c in range(NCORES):
        m = dict(iota=iota)
        for st, et, _ in l1_ets:
            m[f"ptab_{et}"] = tab_bf[st]
        for _, et, _ in l1_ets:
            m[f"w1_{et}"] = _bf(inputs[f"W1_{et}"])
        for _, et, _ in l2_ets:
            m[f"w2_{et}"] = _bf(inputs[f"W2_{et}"])
        for li, LP in ((1, L1), (2, L2)):
            m[f"idx{li}"] = LP.tensors[c]["idx"]
            m[f"rel{li}"] = LP.tensors[c]["rel"]
            m[f"rdeg{li}"] = LP.tensors[c]["rdeg"]
        in_maps.append(m)

    if os.environ.get("KERNEL_SIM", "0") == "1":
        from concourse.bass_interp import MultiCoreSim
        sim = MultiCoreSim(nc, num_cores=NCORES, trace=False,
                           require_finite=False, require_nnan=False)
        cores = list(sim.cores.values())
        for c, core in enumerate(cores):
            for name, arr in in_maps[c].items():
                core.tensor(name)[:] = arr
        sim.simulate(check_with_hw=False, trace_hw=False)

        class _R:
            results = [{"out": np.asarray(core.tensor("out"))} for core in cores]
            exec_time_ns = None
        res = _R()
    else:
        trace = os.environ.get("KERNEL_TRACE", "0") == "1"
        res = run_bass_kernel_spmd(nc, in_maps, core_ids=list(range(NCORES)),
                                   trace=trace, trace_cores=[0] if trace else None)

    pc, pl = dst_map["chemical"]
    out = np.empty((n_nodes["chemical"], D), np.float32)
    for c in range(NCORES):
        mine = np.where(pc == c)[0]
        out[mine] = np.asarray(res.results[c]["out"])[pl[mine]]
    return out, res


def kernel(**inputs):
    n_nodes = {"chemical": inputs["chemical_embed"].shape[0],
               "gene": inputs["gene_embed"].shape[0]}
    if any(np.any(np.asarray(inputs[f"b{k}_{et}"]) != 0)
           for k in (1, 2) for _, et, _ in ETYPES):
        return _np_reference(inputs, n_nodes)
    out, _ = run(inputs, n_nodes)
    return out



# revision 47
# speedup vs baseline: 1.0608x; 1.0608x over previous
"""Trainium2 Bass kernel for nn_NodeClassifier (2-layer hetero-RGCN, mean aggregation).

Strategy (8 NeuronCores, dst-node sharding):
  - Mean-aggregation commutes with the per-etype linear: segmean(h @ W) = segmean(h) @ W,
    so we gather RAW bf16 node rows per edge (dma_gather), segment-sum them via TensorE
    matmuls against an on-device-built 0/1 selection matrix (dst_rel == iota), apply the
    256x256 weights after aggregation, and scale rows by 1/deg at the combine stage.
  - Edges are sharded by destination-node ownership; nodes are dealt to
    (core, window-slot) bins in descending-degree stripes so per-window edge counts
    stay balanced across cores (smaller max-over-core chunk quotas).
  - Chunks are packed at GROUP-of-windows granularity and may span window
    boundaries; a chunk feeds each overlapped window through its own masked
    S column (rel values outside the window never match iota).
  - SWDGE gathers run on 4 queues; the completion-semaphore -> queue map is
    rewritten post-scheduling so every sem is driven by exactly one queue
    (ucode requirement) while adjacent calls alternate queues (desc-gen of
    the 4 Q7 queue contexts runs concurrently: ~2.7x pool throughput).
  - Per-(core, etype) packed layer-1 tables (each core's distinct src rows,
    re-indexed < 32768) kill the int16 lo/hi split for layer 1.  Layer 2
    gathers from the AllGathered h2 tables and keeps the lo/hi split.
  - Vector work is batched (one is_equal S-build per gather call); PSUM
    evacuation, 1/deg scaling (per-partition scale) and leaky-relu run on the
    otherwise-idle Activation engine.
"""
import os
import sys

for _p in ("/opt/trn_rl_repo", "/root/.axon_site/_ro/trn_rl_repo"):
    if os.path.isdir(_p) and _p not in sys.path:
        sys.path.append(_p)

import numpy as np
import ml_dtypes

import concourse.bass as bass
import concourse.bacc as bacc
import concourse.mybir as mybir
import concourse.tile as tile
from concourse.bass_utils import run_bass_kernel_spmd

BF16 = mybir.dt.bfloat16
F32 = mybir.dt.float32
I16 = mybir.dt.int16

ETYPES = [("chemical", "ch2ge", "gene"),
          ("gene", "ge2ch", "chemical"),
          ("chemical", "ch2ch", "chemical"),
          ("gene", "ge2ge", "gene")]
D = 256
NCORES = 8
LO_LIM = 32768
GROUP = int(os.environ.get("KERNEL_GROUP", "3"))   # windows per dma_gather call
NQ = int(os.environ.get("KERNEL_NQ", "4"))   # swdge queues (validated post-compile; >1 falls back unless provably race-free)


def _bf(x):
    return np.ascontiguousarray(np.asarray(x, np.float32)).astype(ml_dtypes.bfloat16)


def _wrap_idx(idx):
    """int16 idx array (len % 128 == 0) -> [128, n/16] wrapped + replicated layout."""
    n = len(idx)
    w = np.zeros((16, n // 16), np.int16)
    ar = np.arange(n)
    w[ar % 16, ar // 16] = idx
    return np.tile(w, (8, 1))


class LayerPrep:
    """Host-side per-layer gather planning with group-granular chunking.

    Edges of each (etype, half) stream are packed per GROUP of windows into
    128-row chunks that may span window boundaries; a chunk contributes to a
    window w through an S column that masks rows of other windows (rel = -1
    never matches iota).  The (chunk, window) incidence is the union over
    cores so the shared SPMD program is valid everywhere.  Produces, per
    core, the gather index / dst_rel / rdeg tensors plus the compile-time
    quota structure (max over cores)."""

    def __init__(self, n_nodes, etlist, srcs, dsts, src_row_of, n_src_rows,
                 dst_map=None):
        self.etlist = etlist
        self.slice_n = {nt: n // NCORES for nt, n in n_nodes.items()}
        self.wpc = {nt: (self.slice_n[nt] + 127) // 128 for nt in n_nodes}
        self.rows_pad = {nt: self.wpc[nt] * 128 for nt in n_nodes}
        self.gquota = {}      # (et, half) -> list[ngrp] of chunk counts
        self.gcols = {}       # (et, half) -> list[ngrp] of S-column counts
        self.win_chunks = {}  # (et, half, grp) -> {w: [(chunk_local, col_local)]}
        self.n_src_rows = n_src_rows
        self.dst_map = dst_map or {}

        percore = [dict() for _ in range(NCORES)]
        for st, et, dt in etlist:
            s, d = srcs[et], dsts[et]
            sn, wpc = self.slice_n[dt], self.wpc[dt]
            ngrp = -(-wpc // GROUP)
            if dt in self.dst_map:
                pc, pl = self.dst_map[dt]
                core_of, loc = pc[d], pl[d]
            else:
                core_of = d // sn
                loc = d - core_of * sn
            win, rel = loc // 128, loc % 128
            rows_spec = src_row_of[et]
            per_core_rows = isinstance(rows_spec, (list, tuple))
            if per_core_rows:
                half = np.zeros(len(s), np.int8)   # packed rows all < LO_LIM
                s_rows = None
            else:
                rows = rows_spec[s]
                half = (rows >= LO_LIM).astype(np.int8)
            deg = np.bincount(d, minlength=n_nodes[dt]).astype(np.float32)
            rdeg_full = 1.0 / np.maximum(deg, 1.0)

            counts = np.zeros((NCORES, wpc, 2), np.int64)
            np.add.at(counts, (core_of, win, half), 1)

            key = core_of.astype(np.int64) * (wpc * 2) + win * 2 + half
            order = np.argsort(key, kind="stable")
            s_src, s_rel = s[order], rel[order]
            if not per_core_rows:
                s_rows = rows[order]
            s_key = key[order]

            for h in (0, 1):
                if per_core_rows and h == 1:
                    self.gquota[(et, 1)] = [0] * ngrp
                    self.gcols[(et, 1)] = [0] * ngrp
                    for g in range(ngrp):
                        self.win_chunks[(et, 1, g)] = {}
                    continue
                gq, gc = [], []
                for g in range(ngrp):
                    ws = range(g * GROUP, min((g + 1) * GROUP, wpc))
                    cg = counts[:, list(ws), h].sum(axis=1)       # per core
                    nch = max(1, -(-int(cg.max()) // 128))
                    gq.append(nch)
                    # union (chunk, window) incidence over cores
                    pairs = set()
                    for c in range(NCORES):
                        off = 0
                        for w in ws:
                            cnt = int(counts[c, w, h])
                            a, b = off, off + cnt
                            j0 = min(a // 128, nch - 1)
                            j1 = min(max(b - 1, a) // 128, nch - 1)
                            for j in range(j0, j1 + 1):
                                pairs.add((j, w))
                            off = b
                    plist = sorted(pairs, key=lambda p: (p[1], p[0]))
                    gc.append(len(plist))
                    self.win_chunks[(et, h, g)] = wc = {}
                    for ci, (j, w) in enumerate(plist):
                        wc.setdefault(w, []).append((j, ci))
                self.gquota[(et, h)] = gq
                self.gcols[(et, h)] = gc

            for c in range(NCORES):
                for h in (0, 1):
                    gq = self.gquota[(et, h)]
                    nch_tot = sum(gq)
                    idx_arr = np.zeros(nch_tot * 128, np.int16)
                    ncol_tot = sum(self.gcols[(et, h)])
                    rel_mat = np.full((ncol_tot, 128), -1.0, np.float32)
                    goff = col0 = 0
                    for g in range(-(-wpc // GROUP)):
                        ws = range(g * GROUP, min((g + 1) * GROUP, wpc))
                        off = 0
                        spans = {}
                        for w in ws:
                            kk = c * (wpc * 2) + w * 2 + h
                            a = np.searchsorted(s_key, kk)
                            b = np.searchsorted(s_key, kk, side="right")
                            cnt = b - a
                            if per_core_rows:
                                vals = rows_spec[c][s_src[a:b]]
                            else:
                                vals = s_rows[a:b] - LO_LIM * h
                            idx_arr[goff + off:goff + off + cnt] = \
                                vals.astype(np.int16)
                            spans[w] = (off, off + cnt, a)
                            off += cnt
                        # fill this core's rel columns for the group's pairs
                        wc = self.win_chunks[(et, h, g)]
                        for w, jlist in wc.items():
                            o0, o1, a = spans[w]
                            for (j, ci) in jlist:
                                lo = max(o0, j * 128)
                                hi = min(o1, (j + 1) * 128)
                                if hi > lo:
                                    rel_mat[col0 + ci, lo - j * 128:hi - j * 128] = \
                                        s_rel[a + (lo - o0):a + (hi - o0)]
                        goff += gq[g] * 128
                        col0 += self.gcols[(et, h)][g]
                    percore[c][(et, h, "idx")] = idx_arr
                    percore[c][(et, h, "rel")] = rel_mat
                pad = np.ones(self.rows_pad[dt], np.float32)
                if dt in self.dst_map:
                    pc, pl = self.dst_map[dt]
                    mine = np.where(pc == c)[0]
                    pad[pl[mine]] = rdeg_full[mine]
                else:
                    pad[:sn] = rdeg_full[c * sn:c * sn + sn]
                percore[c][(et, "rdeg")] = pad.reshape(wpc, 128).T.copy()

        # stream offsets (chunks / S cols / rdeg) in the concatenated tensors
        self.chunk_off, self.col_off, self.rdeg_off = {}, {}, {}
        ch_cur = co_cur = rd_cur = 0
        for st, et, dt in etlist:
            for h in (0, 1):
                self.chunk_off[(et, h)] = ch_cur
                ch_cur += sum(self.gquota[(et, h)])
                self.col_off[(et, h)] = co_cur
                co_cur += sum(self.gcols[(et, h)])
            self.rdeg_off[et] = rd_cur
            rd_cur += self.wpc[dt]
        self.tot_chunks, self.tot_cols, self.tot_rdeg = ch_cur, co_cur, rd_cur

        self.tensors = []
        for c in range(NCORES):
            idx_cols, rdegs = [], []
            rel_mat = np.full((128, self.tot_cols), -1.0, np.float32)
            for st, et, dt in etlist:
                for h in (0, 1):
                    idx_cols.append(_wrap_idx(percore[c][(et, h, "idx")]))
                    rel = percore[c][(et, h, "rel")]
                    co = self.col_off[(et, h)]
                    rel_mat[:, co:co + rel.shape[0]] = rel.T
                rdegs.append(percore[c][(et, "rdeg")])
            self.tensors.append(dict(
                idx=np.concatenate(idx_cols, axis=1),
                rel=rel_mat.astype(ml_dtypes.bfloat16),
                rdeg=np.ascontiguousarray(np.concatenate(rdegs, axis=1)),
            ))


def _np_reference(inputs, n_nodes):
    """Pure-numpy fp32 fallback (used only when biases are nonzero)."""
    def layer(h, Wk, bk):
        agg = {nt: np.zeros((n, D), np.float32) for nt, n in n_nodes.items()}
        for st, et, dt in ETYPES:
            Wh = h[st] @ inputs[f"{Wk}_{et}"] + inputs[f"{bk}_{et}"]
            msg = Wh[inputs[f"src_{et}"]]
            ssum = np.zeros((n_nodes[dt], D), np.float32)
            np.add.at(ssum, inputs[f"dst_{et}"], msg)
            cnt = np.bincount(inputs[f"dst_{et}"], minlength=n_nodes[dt]).astype(np.float32)[:, None]
            agg[dt] += ssum / np.maximum(cnt, 1.0)
        return agg
    h = {"chemical": np.asarray(inputs["chemical_embed"], np.float32),
         "gene": np.asarray(inputs["gene_embed"], np.float32)}
    h = layer(h, "W1", "b1")
    h = {k: np.where(v > 0, v, np.float32(0.01) * v) for k, v in h.items()}
    return layer(h, "W2", "b2")["chemical"]


def _builder(nq, inputs, n_nodes, L1, L2, n_packed):
    l1_ets = ETYPES
    l2_ets = [e for e in ETYPES if e[2] == 'chemical']
    nc = bacc.Bacc("TRN2", target_bir_lowering=False, debug=False,
                   num_devices=NCORES, num_swdge_queues=nq)
    # per-(core, etype) packed source tables for layer 1 (content differs per
    # core via in_maps; all packed rows < LO_LIM so L1 has no hi stream)
    tabs = {et: nc.dram_tensor(f"ptab_{et}", [n_packed[et], D], BF16,
                               kind="ExternalInput")
            for _, et, _ in l1_ets}
    w_in = {(1, et): nc.dram_tensor(f"w1_{et}", [D, D], BF16, kind="ExternalInput")
            for _, et, _ in l1_ets}
    w_in.update({(2, et): nc.dram_tensor(f"w2_{et}", [D, D], BF16, kind="ExternalInput")
                 for _, et, _ in l2_ets})
    dram_in = {}
    for li, LP in ((1, L1), (2, L2)):
        t0 = LP.tensors[0]
        dram_in[(li, "idx")] = nc.dram_tensor(f"idx{li}", list(t0["idx"].shape), I16,
                                              kind="ExternalInput")
        dram_in[(li, "rel")] = nc.dram_tensor(f"rel{li}", list(t0["rel"].shape), BF16,
                                              kind="ExternalInput")
        dram_in[(li, "rdeg")] = nc.dram_tensor(f"rdeg{li}", list(t0["rdeg"].shape), F32,
                                               kind="ExternalInput")
    iota_t = nc.dram_tensor("iota", [128, 128], F32, kind="ExternalInput")
    out_t = nc.dram_tensor("out", [L1.rows_pad["chemical"], D], F32, kind="ExternalOutput")

    h2_slice = {nt: nc.dram_tensor(f"h2s_{nt}", [L1.rows_pad[nt], D], BF16)
                for nt in n_nodes}
    h2_full = {nt: nc.dram_tensor(f"h2f_{nt}", [L1.rows_pad[nt] * NCORES, D], BF16,
                                  addr_space="Shared")
               for nt in n_nodes}

    import contextlib
    with tile.TileContext(nc) as tc, contextlib.ExitStack() as ctx:
        const = ctx.enter_context(tc.tile_pool(name="const", bufs=1))
        iota_f = const.tile([128, 128], F32, tag="iotaf")
        iota_sb = const.tile([128, 1, 128], BF16, tag="iotab")
        nc.sync.dma_start(iota_f[:], iota_t[:])
        nc.vector.tensor_copy(iota_sb[:, 0, :], iota_f[:])
        w_sb = {}
        for key, t in w_in.items():
            # scalar-engine DMA: keeps the weight loads off the Sync FIFO so
            # the first gather's idx load isn't queued behind them
            w = const.tile([128, 2 * D], BF16, tag=f"w_{key[0]}_{key[1]}")
            nc.scalar.dma_start(w[:, 0:D], t[0:128, :])
            nc.scalar.dma_start(w[:, D:2 * D], t[128:256, :])
            w_sb[key] = w
        rel_sb, rdeg_sb = {}, {}
        for li, LP in ((1, L1), (2, L2)):
            r = const.tile(list(LP.tensors[0]["rel"].shape), BF16, tag=f"rel{li}")
            nc.sync.dma_start(r[:], dram_in[(li, "rel")][:])
            rel_sb[li] = r
            g = const.tile(list(LP.tensors[0]["rdeg"].shape), F32, tag=f"rdeg{li}")
            nc.sync.dma_start(g[:], dram_in[(li, "rdeg")][:])
            rdeg_sb[li] = g

        def do_layer(li, LP, gtab, sink, ntype_done=None, prepass_et=None):
            etlist = LP.etlist
            with contextlib.ExitStack() as lctx:
                gp, ip, sp = {}, {}, {}

                def open_stream_pools(stack, ets):
                    for st, et, dt in ets:
                        for h in (0, 1):
                            if sum(LP.gquota[(et, h)]) == 0:
                                continue
                            nb = 3 if (et == prepass_et and h == 0) else \
                                int(os.environ.get("KERNEL_L1BUFS", "2")) \
                                if li == 1 else \
                                int(os.environ.get("KERNEL_L2BUFS", "2")) \
                                if (li == 2 and h == 0) else 2
                            gp[(et, h)] = stack.enter_context(
                                tc.tile_pool(name=f"g{li}{et}{h}", bufs=nb))
                            ip[(et, h)] = stack.enter_context(
                                tc.tile_pool(name=f"i{li}{et}{h}", bufs=2))
                            sp[(et, h)] = stack.enter_context(
                                tc.tile_pool(name=f"s{li}{et}{h}", bufs=2))
                mt_psum = lctx.enter_context(
                    tc.tile_pool(name=f"mtp{li}", bufs=2, space="PSUM"))
                agg_psum = lctx.enter_context(
                    tc.tile_pool(name=f"agp{li}", bufs=2, space="PSUM"))
                mt_pool = lctx.enter_context(tc.tile_pool(name=f"mt{li}", bufs=3))
                cb_pool = lctx.enter_context(tc.tile_pool(name=f"cb{li}", bufs=4))

                call_tiles = {}

                def compute_mt(et, grp, w, pool):
                    """Segment-sum the window's chunks into PSUM, copy to SBUF bf16."""
                    chunks = []
                    for h in (0, 1):
                        for (j, cl) in LP.win_chunks[(et, h, grp)].get(w, []):
                            chunks.append((h, j, cl))
                    mt0 = mt_psum.tile([128, 128], F32, tag="mt0")
                    mt1 = mt_psum.tile([128, 128], F32, tag="mt1")
                    mts = [mt0, mt1]
                    nlast = len(chunks) - 1
                    for ci, (h, ch_l, cl) in enumerate(chunks):
                        gt, stt, _w0 = call_tiles[(et, h, grp)]
                        for fh in (0, 1):
                            nc.tensor.matmul(
                                mts[fh][:],
                                lhsT=gt[:, ch_l, fh * 128:(fh + 1) * 128],
                                rhs=stt[:, cl, :],
                                start=(ci == 0), stop=(ci == nlast))
                    mt_sb = pool.tile([128, 2 * 128], BF16, tag="mtsb")
                    nc.scalar.activation(mt_sb[:, 0:128], mts[0][:],
                                         mybir.ActivationFunctionType.Copy)
                    nc.scalar.activation(mt_sb[:, 128:256], mts[1][:],
                                         mybir.ActivationFunctionType.Copy)
                    return mt_sb

                def wapply(et, mt_sb):
                    ag = agg_psum.tile([128, D], F32, tag="agg")
                    for fh in (0, 1):
                        nc.tensor.matmul(
                            ag[:],
                            lhsT=mt_sb[:, fh * 128:(fh + 1) * 128],
                            rhs=w_sb[(li, et)][:, fh * D:(fh + 1) * D],
                            start=(fh == 0), stop=(fh == 1))
                    return ag

                def issue_call(et, h, dt, grp, qnum):
                    gq = LP.gquota[(et, h)]
                    w0 = grp * GROUP
                    nch = gq[grp]
                    if nch == 0:
                        return
                    chunk_base = LP.chunk_off[(et, h)] + sum(gq[:grp])
                    icol0 = chunk_base * 8          # 128 idx per chunk / 16
                    it = ip[(et, h)].tile([128, nch * 8], I16, tag=f"it{et}{h}")
                    nc.sync.dma_start(it[:], dram_in[(li, "idx")][:, icol0:icol0 + nch * 8])
                    gt = gp[(et, h)].tile([128, nch, D], BF16, tag=f"gt{et}{h}")
                    tab, nrows = gtab[et]
                    base = LO_LIM * h
                    if base >= nrows:
                        base = 0    # half has no real rows; pads (idx 0) only
                    view = tab[base:min(base + LO_LIM, nrows), :]
                    nc.gpsimd.dma_gather(
                        out_ap=gt[:], in_ap=view, idxs_ap=it[:],
                        num_idxs=nch * 128, num_idxs_reg=nch * 128,
                        elem_size=D, single_packet=False, queue_num=qnum)
                    # one batched S-build for the whole call: [128, ncols, 128]
                    ncols = LP.gcols[(et, h)][grp]
                    scol0 = LP.col_off[(et, h)] + sum(LP.gcols[(et, h)][:grp])
                    stt = sp[(et, h)].tile([128, ncols, 128], BF16, tag=f"st{et}{h}")
                    nc.vector.tensor_tensor(
                        out=stt[:],
                        in0=rel_sb[li][:, scol0:scol0 + ncols]
                            .to_broadcast([128, ncols, 128]),
                        in1=iota_sb[:].to_broadcast([128, ncols, 128]),
                        op=mybir.AluOpType.is_equal)
                    call_tiles[(et, h, grp)] = (gt, stt, w0)

                ntypes = sorted({dt for _, _, dt in etlist}, reverse=True)  # gene first: its AllGather overlaps chemical-dst compute
                for nt in ntypes:
                    my_ets = [e for e in etlist if e[2] == nt]
                    with contextlib.ExitStack() as pctx:
                        open_stream_pools(pctx, my_ets)
                        wpc = LP.wpc[nt]
                        ngrp = -(-wpc // GROUP)
                        pre_mts = {}
                        if prepass_et and any(et == prepass_et
                                              for _, et, _ in my_ets):
                            # Pass A: gather + segment-sum for prepass_et only.
                            # Its source table is ready before the other
                            # etypes' (it only depends on the earlier
                            # AllGather), so this work fills the wait for the
                            # later AllGather.
                            pmt_pool = pctx.enter_context(
                                tc.tile_pool(name=f"pm{li}", bufs=wpc))
                            for grp in range(ngrp):
                                for h in (0, 1):
                                    issue_call(prepass_et, h, nt, grp, 0)
                                for w in range(grp * GROUP,
                                               min((grp + 1) * GROUP, wpc)):
                                    pre_mts[w] = compute_mt(prepass_et, grp, w,
                                                            pmt_pool)
                            my_ets = [e for e in my_ets
                                      if e[1] != prepass_et]
                        for grp in range(ngrp):
                            for st_, et, _ in my_ets:
                                for h in (0, 1):
                                    issue_call(et, h, nt, grp, 0)
                            for w in range(grp * GROUP,
                                           min((grp + 1) * GROUP, wpc)):
                                aggs = []
                                if w in pre_mts:
                                    aggs.append((prepass_et,
                                                 wapply(prepass_et,
                                                        pre_mts[w])))
                                for st_, et, _ in my_ets:
                                    mt_sb = compute_mt(et, grp, w, mt_pool)
                                    aggs.append((et, wapply(et, mt_sb)))
                                sink(nt, w, aggs, cb_pool, li, LP)
                    if ntype_done:
                        ntype_done(nt)

        def combine(nt, w, aggs, cb_pool, li, LP):
            acc = None
            for et, ag in aggs:
                col = LP.rdeg_off[et] + w
                t = cb_pool.tile([128, D], F32, tag="cbt")
                nc.scalar.activation(t[:], ag[:],
                                     mybir.ActivationFunctionType.Copy,
                                     scale=rdeg_sb[li][:, col:col + 1])
                if acc is None:
                    acc = t
                else:
                    t2 = cb_pool.tile([128, D], F32, tag="cbt")
                    nc.vector.tensor_tensor(out=t2[:], in0=acc[:], in1=t[:],
                                            op=mybir.AluOpType.add)
                    acc = t2
            if li == 1:
                h2w = cb_pool.tile([128, D], BF16, tag="h2w")
                nc.scalar.activation(h2w[:], acc[:],
                                     mybir.ActivationFunctionType.Lrelu,
                                     alpha=0.01)
                nc.sync.dma_start(h2_slice[nt][w * 128:(w + 1) * 128, :], h2w[:])
            else:
                nc.sync.dma_start(out_t[w * 128:(w + 1) * 128, :], acc[:])

        def ag_ntype(nt):
            nc.gpsimd.collective_compute(
                "AllGather", mybir.AluOpType.bypass,
                replica_groups=[list(range(NCORES))],
                ins=[h2_slice[nt].ap().opt()],
                outs=[h2_full[nt].ap().opt()])

        gtab1 = {et: (tabs[et], n_packed[et]) for st, et, _ in l1_ets}
        gtab2 = {et: (h2_full[st], L1.rows_pad[st] * NCORES) for st, et, _ in l2_ets}
        do_layer(1, L1, gtab1, combine, ntype_done=ag_ntype)
        do_layer(2, L2, gtab2, combine)

    if nq > 1:
        # Re-derive each gather's queue from its scheduler-assigned completion
        # semaphore: sem -> queue is then 1:1 (the ucode constraint) and
        # per-sem increments stay in-order (same queue => FIFO).  The map
        # keeps consecutive sem lanes on different queues (adjacent calls
        # pipeline across queues) while pairing each heavy lo-half lane with
        # a light hi-half lane per queue for byte balance.
        lane_q = {0: 0, 1: 1, 2: 2, 3: 3, 4: 1, 5: 0, 6: 3, 7: 2}
        gathers = []
        for bb in nc.m.functions[0].blocks:
            for ins in bb.instructions:
                if isinstance(ins, mybir.InstDMAGatherAnt) and ins.sync_info:
                    sems = [u.id for u in ins.sync_info.on_update
                            if u.sync_type == "semaphore"]
                    if sems:
                        gathers.append((ins, sems[0]))
        order = {s: i for i, s in enumerate(dict.fromkeys(s for _, s in gathers))}
        for ins, s in gathers:
            ins.queue_num = lane_q[order[s] % 8] % nq

    nc.compile()

    return nc


def run(inputs, n_nodes):
    srcs = {et: np.asarray(inputs[f"src_{et}"]) for _, et, _ in ETYPES}
    dsts = {et: np.asarray(inputs[f"dst_{et}"]) for _, et, _ in ETYPES}
    l1_ets = ETYPES
    l2_ets = [e for e in ETYPES if e[2] == "chemical"]

    # Striped dst-sharding: deal degree-sorted nodes round-robin across
    # (core, window-slot) so per-(etype, window) edge counts flatten across
    # cores and windows -> smaller max-over-cores chunk quotas.
    dst_map = {}
    for nt, n in n_nodes.items():
        deg = np.zeros(n, np.int64)
        for _, et, dt in ETYPES:
            if dt == nt:
                deg += np.bincount(dsts[et], minlength=n)
        order = np.argsort(-deg, kind="stable")
        pc = np.empty(n, np.int64)
        pl = np.empty(n, np.int64)
        r = np.arange(n)
        pc[order] = r % NCORES
        pl[order] = r // NCORES
        dst_map[nt] = (pc, pl)

    # layer-1 gathers read the raw embed tables at node index (lo/hi halves;
    # per-core packed tables measured slower: coarser calls pipeline worse)
    tab_bf = {"chemical": _bf(inputs["chemical_embed"]), "gene": _bf(inputs["gene_embed"])}
    ident = {nt: np.arange(n, dtype=np.int64) for nt, n in n_nodes.items()}
    row_of1 = {et: ident[st] for st, et, _ in l1_ets}
    n_packed = {et: n_nodes[st] for st, et, _ in l1_ets}
    L1 = LayerPrep(n_nodes, l1_ets, srcs, dsts, row_of1, dict(n_nodes),
                   dst_map=dst_map)

    # layer-2 rows live in the AllGathered (per-core padded) tables
    row_of2, n_rows2 = {}, {}
    for nt in n_nodes:
        pc, pl = dst_map[nt]
        row_of2[nt] = pc * L1.rows_pad[nt] + pl
        n_rows2[nt] = L1.rows_pad[nt] * NCORES
    L2 = LayerPrep(n_nodes, l2_ets, srcs, dsts,
                   {et: row_of2[st] for st, et, _ in l2_ets}, n_rows2,
                   dst_map=dst_map)
    iota = np.tile(np.arange(128, dtype=np.float32)[None, :], (128, 1))

    def _swdge_queues_ok(nc_):
        """Each SWDGE completion semaphore must be driven by exactly one queue
        (ucode locks a sem to the first queue that uses it)."""
        qmap = {}
        for bb in nc_.m.functions[0].blocks:
            for ins in bb.instructions:
                if isinstance(ins, mybir.InstDMAGatherAnt) and ins.sync_info:
                    for u in ins.sync_info.on_update:
                        if u.sync_type == "semaphore":
                            qmap.setdefault(u.id, set()).add(ins.queue_num)
        return all(len(v) == 1 for v in qmap.values())

    nc = None
    for nq_try in [NQ] + [q for q in (2, 1) if q < NQ]:
        nc = _builder(nq_try, inputs, n_nodes, L1, L2, n_packed)
        if _swdge_queues_ok(nc):
            print(f"[kernel] using num_swdge_queues={nq_try}")
            break
        print(f"[kernel] queue collision at nq={nq_try}, falling back")
    assert nc is not None

    in_maps = []
    for c in range(NCORES):
        m = dict(iota=iota)
        for st, et, _ in l1_ets:
            m[f"ptab_{et}"] = tab_bf[st]
        for _, et, _ in l1_ets:
            m[f"w1_{et}"] = _bf(inputs[f"W1_{et}"])
        for _, et, _ in l2_ets:
            m[f"w2_{et}"] = _bf(inputs[f"W2_{et}"])
        for li, LP in ((1, L1), (2, L2)):
            m[f"idx{li}"] = LP.tensors[c]["idx"]
            m[f"rel{li}"] = LP.tensors[c]["rel"]
            m[f"rdeg{li}"] = LP.tensors[c]["rdeg"]
        in_maps.append(m)

    if os.environ.get("KERNEL_SIM", "0") == "1":
        from concourse.bass_interp import MultiCoreSim
        sim = MultiCoreSim(nc, num_cores=NCORES, trace=False,
                           require_finite=False, require_nnan=False)
        cores = list(sim.cores.values())
        for c, core in enumerate(cores):
            for name, arr in in_maps[c].items():
                core.tensor(name)[:] = arr
        sim.simulate(check_with_hw=False, trace_hw=False)

        class _R:
            results = [{"out": np.asarray(core.tensor("out"))} for core in cores]
            exec_time_ns = None
        res = _R()
    else:
        trace = os.environ.get("KERNEL_TRACE", "0") == "1"
        res = run_bass_kernel_spmd(nc, in_maps, core_ids=list(range(NCORES)),
                                   trace=trace, trace_cores=[0] if trace else None)

    pc, pl = dst_map["chemical"]
    out = np.empty((n_nodes["chemical"], D), np.float32)
    for c in range(NCORES):
        mine = np.where(pc == c)[0]
        out[mine] = np.asarray(res.results[c]["out"])[pl[mine]]
    return out, res


def kernel(**inputs):
    n_nodes = {"chemical": inputs["chemical_embed"].shape[0],
               "gene": inputs["gene_embed"].shape[0]}
    if any(np.any(np.asarray(inputs[f"b{k}_{et}"]) != 0)
           for k in (1, 2) for _, et, _ in ETYPES):
        return _np_reference(inputs, n_nodes)
    out, _ = run(inputs, n_nodes)
    return out

